# revision 61
# baseline (speedup 1.0000x reference)
"""Trainium2 Bass kernel for nn_ExpSelfAttention (dense transformer block).

Math (per batch item b, all f32 data):
    y  = LN(x; g1, beta1);  z = y @ w_lin.T + b_lin
    attn = W @ z            (W = causal exp-decay matrix, alpha=0.9)
    x2 = x + attn
    y2 = LN(x2; g2, beta2); h = relu(y2 @ w1.T + b1)
    out = x2 + h @ w2.T + b2

Sharding: data parallel over batch (16 / 8 cores = 2 per core); weights and
the (input-independent) decay-matrix blocks replicated. No collectives.

Precision plan (rel-err budget 2e-2, this lands ~5e-3):
  - FFN matmuls in fp8-e4m3 with MatmulPerfMode.DoubleRow (packs two
    contraction rows per PE cell: 0.5 cyc/output-row and K=256 per
    instruction -> 4x the f32r FLOP rate). y2/h/w1/w2 quantized to fp8.
  - Projection in bf16 (feeds the decay mixing whose output dominates the
    result -- fp8 there would blow the error budget); mixing in f32r.
  - x2 residual held in bf16; final output assembled in f32.

Engine balance (per-batch-item busy, approx): PE 55us (proj 14, mix 7,
FFN 27, transposes 7), DVE 53us (bn_stats/aggr, z+x2 PSUM evict-adds,
xT evict copies, 3/16 of the relus), Act 50us (relu 13/16, y2T evicts,
final out copies, sqrt), Pool/gpsimd 44us (both LN normalizes, x2+b2).
b2 and x2 are folded into the FFN2 PSUM accumulation via an identity-
weight matmul so the output eviction is a pure Act copy.

All big weights are pre-cast on the host and passed as fp8/bf16 DRAM
parameters (halves weight DMA traffic; no on-chip cast pass).
"""

import sys
from contextlib import ExitStack

for _p in ("/opt/trn_rl_repo", "/opt/pypackages"):
    if _p not in sys.path:
        sys.path.insert(0, _p)

import numpy as np
import ml_dtypes

import concourse.bass as bass
import concourse.mybir as mybir
import concourse.tile as tile
from concourse import bacc
from concourse.bass_utils import run_bass_kernel_spmd
from concourse.masks import make_identity

ALPHA, EPS = 0.9, 1e-5
S, B, D, FF = 2048, 16, 512, 2048
NCORES = 8
BL = B // NCORES            # batch items per core
T = 128                     # token tile
CB = 4                      # token tiles per chunk
NBLK = S // T               # 16
NCHUNK = NBLK // CB         # 4
NFT = FF // 128             # 16 f-tiles
KD = D // 128               # 4 d-tiles
NLAG = 1                    # decay lag blocks kept (lag>=2 < 2e-12 relative)
DVE_RELUS = (7, 15)          # these relu f-tiles run on DVE, rest on Act

F32 = mybir.dt.float32
F32R = mybir.dt.float32r
BF16 = mybir.dt.bfloat16
F8 = mybir.dt.float8e4
AF = mybir.ActivationFunctionType
ALU = mybir.AluOpType
DR = mybir.MatmulPerfMode.DoubleRow

NP_F8 = ml_dtypes.float8_e4m3
NP_BF16 = ml_dtypes.bfloat16


def _host_consts():
    """Decay-matrix derived constants, f64 -> f32 (mirrors reference)."""
    i = np.arange(S, dtype=np.float64)
    diff = i[:, None] - i[None, :]
    with np.errstate(under="ignore"):
        W = np.where(diff >= 0, ALPHA ** (diff + 1), 0.0)
        W = W + np.diag(1.0 - W.sum(axis=1))
        W = W.astype(np.float32)
        blocks = [
            np.ascontiguousarray(W[c * T : (c + 1) * T, c * T : (c + 1) * T].T)
            for c in range(NBLK)
        ]
        uniq, idx = [], []
        for blk in blocks:
            for j, u in enumerate(uniq):
                if np.array_equal(blk, u):
                    idx.append(j)
                    break
            else:
                idx.append(len(uniq))
                uniq.append(blk)
        wblkT = np.stack(uniq)  # [NU, T, T]
        lags = []
        for l in range(1, NLAG + 1):
            L = W[l * T : (l + 1) * T, 0:T]
            for i0 in range(l * T, S, T):
                assert np.array_equal(W[i0 : i0 + T, i0 - l * T : i0 - (l - 1) * T], L)
            lags.append(np.ascontiguousarray(L.T))
        wlagT = np.stack(lags)  # [NLAG, T, T]
    return wblkT.astype(np.float32), idx, wlagT.astype(np.float32)


_WBLKT, _BLKIDX, _WLAGT = _host_consts()
NU = _WBLKT.shape[0]

_NC_CACHE = {}


def build_nc():
    key = 0
    if key in _NC_CACHE:
        return _NC_CACHE[key]
    nc = bacc.Bacc()

    x_d = nc.declare_dram_parameter("x", [S, BL, D], F32, isOutput=False)
    wp_d = nc.declare_dram_parameter("wp", [D, D], BF16, isOutput=False)
    zb_d = nc.declare_dram_parameter("zb", [D], F32, isOutput=False)
    w1t_d = nc.declare_dram_parameter("w1t", [D, FF], F8, isOutput=False)
    hb_d = nc.declare_dram_parameter("hb", [FF], F32, isOutput=False)
    w2t_d = nc.declare_dram_parameter("w2t", [FF, D], F8, isOutput=False)
    b2_d = nc.declare_dram_parameter("b2", [D], F32, isOutput=False)
    wblk_d = nc.declare_dram_parameter("wblk", [NU, T, T], F32, isOutput=False)
    wlag_d = nc.declare_dram_parameter("wlag", [NLAG, T, T], F32, isOutput=False)
    out_d = nc.declare_dram_parameter("out", [S, BL, D], F32, isOutput=True)

    with tile.TileContext(nc) as tc, ExitStack() as ctx:
        pool = lambda name, bufs, **kw: ctx.enter_context(
            tc.tile_pool(name=name, bufs=bufs, **kw)
        )
        wgt = pool("wgt", 1)
        stage = pool("stage", 1)
        xin = pool("xin", 10)
        lnp = pool("ln", 5)
        yppp = pool("ypp", 4)
        xtp = pool("xt", 10)
        y2tp = pool("y2t", 3)
        zp = pool("z", 12)
        x2p = pool("x2", 10)
        hp = pool("h", 3)
        outp = pool("outp", 4)
        # single full-bank PSUM rotation; transposes write through bitcast
        # views so bf16/fp8 tiles share the same bank pool
        psmm = pool("psmm", 8, space="PSUM")

        # ---------------- one-time setup ----------------
        xpre = {}

        def preload_x(i, only=None):
            b, c = steps[i]
            tiles = xpre.get(i, [])
            for t in range(len(tiles), CB):
                if only is not None and t > only:
                    break
                s0 = (c * CB + t) * T
                xt = xin.tile([128, D], F32, tag="x")
                nc.sync.dma_start(xt[:], x_d.ap()[s0 : s0 + T, b, :])
                tiles.append(xt)
            xpre[i] = tiles

        # DMA order matters: the shared DMA device drains FIFO, so x(0) and
        # the weights the first iteration blocks on (zb, wp, wblk) go first.
        steps = [(b, c) for b in range(BL) for c in range(NCHUNK)]
        preload_x(0, only=1)  # first two x tiles, then weights step 0 blocks on
        # projection weight: bf16 straight from DRAM
        wp_r = wgt.tile([128, KD, D], BF16, tag="wp")
        nc.sync.dma_start(wp_r[:], wp_d.ap().rearrange("(kd p) e -> p kd e", p=128))
        zb_bc = wgt.tile([128, D], F32, tag="zb")
        nc.sync.dma_start(
            zb_bc[:], bass.AP(tensor=zb_d, offset=0, ap=[[0, 128], [1, D]])
        )
        preload_x(0)
        # mixing matrices: f32 DRAM -> resident f32r via casting DMA (SWDGE)
        wblk_r = wgt.tile([128, NU, T], F32R, tag="wblk")
        nc.gpsimd.dma_start(wblk_r[:], wblk_d.ap().rearrange("b j r -> j b r"))
        wlag_r = wgt.tile([128, NLAG, T], F32R, tag="wlag")
        nc.gpsimd.dma_start(wlag_r[:], wlag_d.ap().rearrange("b j r -> j b r"))
        preload_x(1)
        b2_bc = wgt.tile([128, D], F32, tag="b2")
        nc.sync.dma_start(
            b2_bc[:], bass.AP(tensor=b2_d, offset=0, ap=[[0, 128], [1, D]])
        )
        hb_sb = wgt.tile([128, NFT], F32, tag="hb")
        nc.sync.dma_start(
            hb_sb[:], bass.AP(tensor=hb_d, offset=0, ap=[[1, 128], [128, NFT]])
        )
        ident_f = stage.tile([128, 128], F32, tag="ident_f")
        make_identity(nc, ident_f[:])
        identB = wgt.tile([128, 128], BF16, tag="identB")
        nc.vector.tensor_copy(identB[:], ident_f[:])
        eps_t = wgt.tile([128, 1], F32, tag="eps")
        nc.vector.memset(eps_t[:], EPS)
        # tiny dummy activation: triggers the one-time activation-table load
        # while the pipeline is still waiting on the first x DMAs
        warm_t = wgt.tile([128, 1], F32, tag="warm")
        nc.scalar.activation(warm_t[:], eps_t[:], AF.Sqrt, bias=eps_t[:], scale=1.0)
        b2b = wgt.tile([128, D], BF16, tag="b2b")
        nc.vector.tensor_copy(b2b[:], b2_bc[:])

        # ---------------- helpers ----------------
        def ln_stats(xts, tag, per_tile=False):
            """4 tiles -> (mv4 [128,4,2], r4 [128,4]).

            batched (default): one sqrt+recip over all 4 tiles (fewer Act
            instrs); per_tile: sqrt/recip per tile so tile 0's normalize can
            start before tile 3's stats land (pipeline fill/drain).
            """
            mv4 = lnp.tile([128, CB, 2], F32, tag=f"mv4{tag}")
            r4 = lnp.tile([128, CB], F32, tag=f"r4{tag}")
            for t in range(CB):
                st = lnp.tile([128, 6], F32, tag=f"bnst{tag}")
                nc.vector.bn_stats(st[:], xts[t][:])
                nc.vector.bn_aggr(mv4[:, t, :], st[:])
                if per_tile:
                    nc.scalar.activation(
                        r4[:, t : t + 1], mv4[:, t, 1:2], AF.Sqrt,
                        bias=eps_t[:], scale=1.0,
                    )
                    nc.vector.reciprocal(r4[:, t : t + 1], r4[:, t : t + 1])
            if not per_tile:
                nc.scalar.activation(
                    r4[:], mv4[:, :, 1], AF.Sqrt, bias=eps_t[:], scale=1.0
                )
                nc.vector.reciprocal(r4[:], r4[:])
            return mv4, r4

        # ---------------- main pipeline ----------------
        zall = {b: [] for b in range(BL)}
        a_out, b_out = {}, {}

        def stage_ln1(i, per_tile=False, interleave_zev=False):
            """LN1 + transpose + projection, tile-interleaved on the PE."""
            b, c = steps[i]
            if i not in xpre:
                preload_x(i)
            xts = xpre.pop(i)
            if per_tile:
                # stats emitted inside the tile loop: tile t's full chain
                # completes while tile t+1's x DMA is still in flight
                mv4 = lnp.tile([128, CB, 2], F32, tag="mv4a")
                r4 = lnp.tile([128, CB], F32, tag="r4a")
            else:
                mv4, r4 = ln_stats(xts, "a")
            xT = []
            pzs = []

            def proj_tile(t):
                pz = psmm.tile([128, D], F32, tag="mm")
                for kd in range(KD):
                    nc.tensor.matmul(
                        pz[:],
                        xT[t][:, kd, :],
                        wp_r[:, kd, :],
                        start=(kd == 0),
                        stop=(kd == KD - 1),
                    )
                pzs.append(pz)
                if interleave_zev:
                    zt = zp.tile([128, D], F32R, tag="z")
                    nc.vector.tensor_add(zt[:], pz[:], zb_bc[:])
                    zall[b].append(zt)

            # one-tile lag: proj(t-1) is emitted after transpose(t), so the
            # PE runs proj(t-1) while tile t's evict copy (DVE) completes
            for t in range(CB):
                if per_tile:
                    st = lnp.tile([128, 6], F32, tag="bnsta")
                    nc.vector.bn_stats(st[:], xts[t][:])
                    nc.vector.bn_aggr(mv4[:, t, :], st[:])
                    nc.scalar.activation(
                        r4[:, t : t + 1], mv4[:, t, 1:2], AF.Sqrt,
                        bias=eps_t[:], scale=1.0,
                    )
                    nc.vector.reciprocal(r4[:, t : t + 1], r4[:, t : t + 1])
                ypp = yppp.tile([128, D], BF16, tag="ypp")
                nc.gpsimd.tensor_scalar(
                    out=ypp[:],
                    in0=xts[t][:],
                    scalar1=mv4[:, t, 0:1],
                    scalar2=r4[:, t : t + 1],
                    op0=ALU.subtract,
                    op1=ALU.mult,
                )
                ptb = psmm.tile([128, D], F32, tag="mm")
                pt = ptb[:].bitcast(BF16)  # [128, 1024] view, use cols 0..511
                for kd in range(KD):
                    nc.tensor.transpose(
                        pt[:, kd * 128 : (kd + 1) * 128],
                        ypp[:, kd * 128 : (kd + 1) * 128],
                        identB[:],
                    )
                xTt = xtp.tile([128, KD, 128], BF16, tag="xT")
                nc.vector.tensor_copy(
                    xTt[:], pt[:, 0:D].rearrange("p (a b) -> p a b", b=128)
                )
                xT.append(xTt)
                if t >= 1:
                    proj_tile(t - 1)
            proj_tile(CB - 1)
            a_out[i] = (xts, xT)
            b_out[i] = pzs

        def stage_zev(i):
            b, c = steps[i]
            for t in range(CB):
                zt = zp.tile([128, D], F32R, tag="z")
                nc.vector.tensor_add(zt[:], b_out[i][t][:], zb_bc[:])
                zall[b].append(zt)
            del b_out[i]

        stage_ln1(0, per_tile=True, interleave_zev=True)
        del b_out[0]

        # big fp8 weights: DMA'd in chunks interleaved with the early
        # pipeline so x loads aren't blocked behind the weight traffic.
        w1t_r = wgt.tile([128, KD, FF], F8, tag="w1t")
        w2t_r = wgt.tile([128, NFT, D], F8, tag="w2t")
        w1t_ap = w1t_d.ap().rearrange("(kd p) f -> p kd f", p=128)
        w2t_ap = w2t_d.ap().rearrange("(kf p) d -> p kf d", p=128)
        wload = [
            lambda kd2=kd2: nc.sync.dma_start(
                w1t_r[:, 2 * kd2 : 2 * kd2 + 2, :], w1t_ap[:, 2 * kd2 : 2 * kd2 + 2, :]
            )
            for kd2 in range(KD // 2)
        ] + [
            lambda f8=f8: nc.sync.dma_start(
                w2t_r[:, 8 * f8 : 8 * f8 + 8, :], w2t_ap[:, 8 * f8 : 8 * f8 + 8, :]
            )
            for f8 in range(2)
        ]
        wload.reverse()  # pop() from the front
        wload.pop()()

        def ffn1_part(y2T, ht, fts, dve_relus=DVE_RELUS):
            """fp8 DoubleRow FFN1 + relu (split Act/DVE) for given f-tiles."""
            for ft in fts:
                ph = psmm.tile([128, CB * 128], F32, tag="mm")
                for j in range(KD // 2):
                    nc.tensor.matmul(
                        ph[:],
                        w1t_r[:, 2 * j : 2 * j + 2, ft * 128 : (ft + 1) * 128],
                        y2T[:, 2 * j : 2 * j + 2, :],
                        start=(j == 0),
                        stop=(j == KD // 2 - 1),
                        perf_mode=DR,
                    )
                if ft in dve_relus:
                    nc.vector.tensor_scalar(
                        out=ht[:, ft, :],
                        in0=ph[:],
                        scalar1=hb_sb[:, ft : ft + 1],
                        scalar2=0.0,
                        op0=ALU.add,
                        op1=ALU.max,
                    )
                else:
                    nc.scalar.activation(
                        ht[:, ft, :],
                        ph[:],
                        AF.Relu,
                        bias=hb_sb[:, ft : ft + 1],
                        scale=1.0,
                    )

        def ffn2_part(ht, x2pbs, b, c, ts):
            """fp8 DoubleRow FFN2 + (x2+b2) via identity matmul -> out DMA."""
            for t in ts:
                s0 = (c * CB + t) * T
                po = psmm.tile([128, D], F32, tag="mm")
                for j in range(NFT // 2):
                    nc.tensor.matmul(
                        po[:],
                        ht[:, 2 * j : 2 * j + 2, t * 128 : (t + 1) * 128],
                        w2t_r[:, 2 * j : 2 * j + 2, :],
                        start=(j == 0),
                        stop=False,
                        perf_mode=DR,
                    )
                nc.tensor.matmul(
                    po[:], identB[:], x2pbs[t][:], start=False, stop=True
                )
                ot = outp.tile([128, D], F32, tag="o")
                nc.scalar.activation(ot[:], po[:], AF.Copy)
                nc.sync.dma_start(out_d.ap()[s0 : s0 + T, b, :], ot[:])

        # Software pipeline, one-step-deep FFN deferral: iteration i runs
        # step i's mixing/LN2 and step i+1's LN1/projection, with step i-1's
        # FFN matmuls emitted where the PE would otherwise stall on the LN
        # stat chains (DVE/Act/Pool latency).
        ffn_prev = None
        for i, (b, c) in enumerate(steps):
            xts, _ = a_out.pop(i)
            # --- mixing (banded decay matmul) ---
            pms = []
            for t in range(CB):
                blk = c * CB + t
                nmix = 1 + min(blk, NLAG)
                pm = psmm.tile([128, D], F32, tag="mm")
                nc.tensor.matmul(
                    pm[:],
                    wblk_r[:, _BLKIDX[blk], :],
                    zall[b][blk][:],
                    start=True,
                    stop=(nmix == 1),
                )
                for l in range(1, nmix):
                    nc.tensor.matmul(
                        pm[:],
                        wlag_r[:, l - 1, :],
                        zall[b][blk - l][:],
                        start=False,
                        stop=(l == nmix - 1),
                    )
                pms.append(pm)
            if i + 1 < len(steps):
                preload_x(i + 1)
            # --- x2 = x + attn (DVE, bf16) with tile-interleaved LN2 stats
            # (per-tile sqrt/recip so norm2(t0) starts ~4us earlier than a
            # batched chain would allow) ---
            x2ts = []
            mv4b = lnp.tile([128, CB, 2], F32, tag="mv4b")
            r4b = lnp.tile([128, CB], F32, tag="r4b")
            for t in range(CB):
                x2t = x2p.tile([128, D], BF16, tag="x2")
                nc.vector.tensor_add(x2t[:], pms[t][:], xts[t][:])
                x2ts.append(x2t)
                st = lnp.tile([128, 6], F32, tag="bnstb")
                nc.vector.bn_stats(st[:], x2t[:])
                nc.vector.bn_aggr(mv4b[:, t, :], st[:])
                if ffn_prev is None:
                    # iteration 0: per-tile sqrt so norm2(t0) starts early
                    nc.scalar.activation(
                        r4b[:, t : t + 1], mv4b[:, t, 1:2], AF.Sqrt,
                        bias=eps_t[:], scale=1.0,
                    )
                    nc.vector.reciprocal(r4b[:, t : t + 1], r4b[:, t : t + 1])

            def ln2_sqrt():
                # batched sqrt+recip, emitted after the first relus so Act
                # starts the relu drain (which gates FFN1's PSUM banks) first
                nc.scalar.activation(
                    r4b[:], mv4b[:, :, 1], AF.Sqrt, bias=eps_t[:], scale=1.0
                )
                nc.vector.reciprocal(r4b[:], r4b[:])
            # --- LN2 per-tile emitter (norm2 Pool, T2 PE, evict Act/DVE) ---
            y2T = y2tp.tile([128, KD, CB * 128], F8, tag="y2T")
            y2pps = {}

            def ln2_norm(t):
                # bf16 through the transpose (the BIR verifier rejects fp8
                # transpose outputs with unit stride); fp8 cast happens at
                # the eviction copy
                y2pp = yppp.tile([128, D], BF16, tag="y2pp")
                nc.gpsimd.tensor_scalar(
                    out=y2pp[:],
                    in0=x2ts[t][:],
                    scalar1=mv4b[:, t, 0:1],
                    scalar2=r4b[:, t : t + 1],
                    op0=ALU.subtract,
                    op1=ALU.mult,
                )
                y2pps[t] = y2pp

            def ln2_tile(t):
                if t not in y2pps:
                    ln2_norm(t)
                y2pp = y2pps.pop(t)
                pt2b = psmm.tile([128, D], F32, tag="mm")
                pt2 = pt2b[:].bitcast(BF16)  # [128, 1024] view, use cols 0..511
                for kd in range(KD):
                    nc.tensor.transpose(
                        pt2[:, kd * 128 : (kd + 1) * 128],
                        y2pp[:, kd * 128 : (kd + 1) * 128],
                        identB[:],
                    )
                if i + 1 == len(steps):
                    # last iteration: Act is clogged with this iteration's
                    # relus/outs -- evict on (idle) DVE so the epilogue FFN
                    # isn't stalled behind them
                    nc.vector.tensor_copy(
                        y2T[:, :, t * 128 : (t + 1) * 128],
                        pt2[:, 0:D].rearrange("p (a b) -> p a b", b=128),
                    )
                else:
                    nc.scalar.activation(
                        y2T[:, :, t * 128 : (t + 1) * 128],
                        pt2[:, 0:D].rearrange("p (a b) -> p a b", b=128),
                        AF.Copy,
                    )

            # --- previous step's FFN: chunky ready-to-run PE work covering
            # this step's LN2 chain and next step's LN1 chain ---
            if ffn_prev is not None:
                ht_prev = hp.tile([128, NFT, CB * 128], F8, tag="h")
                ffn1_part(ffn_prev[0], ht_prev, range(0, 4))
                ln2_sqrt()
                ffn1_part(ffn_prev[0], ht_prev, range(4, NFT))
                if i + 1 == len(steps):
                    # last iteration: run LN2 (whose evicts gate the epilogue
                    # FFN) before ffn2 so the DVE evicts overlap it
                    for t in range(CB):
                        ln2_tile(t)
                    ffn2_part(ht_prev, ffn_prev[1], ffn_prev[2], ffn_prev[3],
                              range(CB))
                else:
                    ffn2_part(ht_prev, ffn_prev[1], ffn_prev[2], ffn_prev[3],
                              range(CB))
                    for t in range(CB):
                        ln2_tile(t)
            else:
                # iteration 0 has no deferred FFN to cover the LN2 chain;
                # run the next step's LN1/projection there instead. The LN2
                # normalizes go first on the Pool queue (they gate T2), while
                # the PE queue gets T1/proj before T2.
                for t in range(CB):
                    ln2_norm(t)
                if i + 1 < len(steps):
                    stage_ln1(i + 1, per_tile=True)
                for t in range(CB):
                    ln2_tile(t)
            if wload:
                wload.pop()()
            # --- next step's LN1 ---
            if ffn_prev is not None and i + 1 < len(steps):
                stage_ln1(i + 1)
            # x2+b2 (Pool) late: only needed by next iteration's ffn2
            x2pbs = []
            for t in range(CB):
                x2pb = x2p.tile([128, D], BF16, tag="x2pb")
                nc.gpsimd.tensor_add(x2pb[:], x2ts[t][:], b2b[:])
                x2pbs.append(x2pb)
            if wload:
                wload.pop()()
            if i + 1 < len(steps):
                stage_zev(i + 1)
            if wload:
                wload.pop()()
            ffn_prev = (y2T, x2pbs, b, c)
        # epilogue: last step's FFN; relu split 50/50 so neither engine's
        # queue becomes the drain tail
        ht_prev = hp.tile([128, NFT, CB * 128], F8, tag="h")
        ffn1_part(ffn_prev[0], ht_prev, range(NFT),
                  dve_relus=(1, 3, 5, 7, 9, 11, 13, 15))
        ffn2_part(ht_prev, ffn_prev[1], ffn_prev[2], ffn_prev[3], range(CB))

    nc.compile()
    _NC_CACHE[key] = nc
    return nc


def _prep_inputs(x, w_lin, b_lin, w1, b1, w2, b2, g1, beta1, g2, beta2):
    f32 = np.float32
    wp = np.ascontiguousarray(w_lin.T * g1[:, None]).astype(NP_BF16)
    zb = (w_lin.astype(np.float64) @ beta1.astype(np.float64) + b_lin).astype(f32)
    w1t = np.ascontiguousarray(w1.T * g2[:, None]).astype(NP_F8)
    hb = (w1.astype(np.float64) @ beta2.astype(np.float64) + b1).astype(f32)
    w2t = np.ascontiguousarray(w2.T).astype(NP_F8)
    shared = {
        "wp": wp,
        "zb": zb,
        "w1t": w1t,
        "hb": hb,
        "w2t": w2t,
        "b2": b2.astype(f32),
        "wblk": _WBLKT,
        "wlag": _WLAGT,
    }
    in_maps = []
    for cc in range(NCORES):
        m = dict(shared)
        m["x"] = np.ascontiguousarray(x[:, cc * BL : (cc + 1) * BL, :]).astype(f32)
        in_maps.append(m)
    return in_maps


def kernel(**inputs):
    nc = build_nc()
    in_maps = _prep_inputs(**inputs)
    res = run_bass_kernel_spmd(nc, in_maps, list(range(NCORES)))
    out = np.concatenate([r["out"] for r in res.results], axis=1)
    return out.astype(np.float32)


if __name__ == "__main__":
    rng = np.random.default_rng(0)
    demo = {
        "x": rng.standard_normal((S, B, D)).astype(np.float32),
        "w_lin": rng.standard_normal((D, D)).astype(np.float32) * D**-0.5,
        "b_lin": rng.standard_normal((D,)).astype(np.float32) * 0.01,
        "w1": rng.standard_normal((FF, D)).astype(np.float32) * D**-0.5,
        "b1": rng.standard_normal((FF,)).astype(np.float32) * 0.01,
        "w2": rng.standard_normal((D, FF)).astype(np.float32) * FF**-0.5,
        "b2": rng.standard_normal((D,)).astype(np.float32) * 0.01,
        "g1": np.ones(D, np.float32),
        "beta1": np.zeros(D, np.float32),
        "g2": np.ones(D, np.float32),
        "beta2": np.zeros(D, np.float32),
    }
    out = kernel(**demo)
    print("ok", out.shape, out.dtype)


# revision 62
# speedup vs baseline: 1.0157x; 1.0157x over previous
"""Trainium2 Bass kernel for nn_ExpSelfAttention (dense transformer block).

Math (per batch item b, all f32 data):
    y  = LN(x; g1, beta1);  z = y @ w_lin.T + b_lin
    attn = W @ z            (W = causal exp-decay matrix, alpha=0.9)
    x2 = x + attn
    y2 = LN(x2; g2, beta2); h = relu(y2 @ w1.T + b1)
    out = x2 + h @ w2.T + b2

Sharding: data parallel over batch (16 / 8 cores = 2 per core); weights and
the (input-independent) decay-matrix blocks replicated. No collectives.

Precision plan (rel-err budget 2e-2, this lands ~5e-3):
  - FFN matmuls in fp8-e4m3 with MatmulPerfMode.DoubleRow (packs two
    contraction rows per PE cell: 0.5 cyc/output-row and K=256 per
    instruction -> 4x the f32r FLOP rate). y2/h/w1/w2 quantized to fp8.
  - Projection in bf16 (feeds the decay mixing whose output dominates the
    result -- fp8 there would blow the error budget); mixing in f32r.
  - x2 residual held in bf16; final output assembled in f32.

Engine balance (per-batch-item busy, approx): PE 55us (proj 14, mix 7,
FFN 27, transposes 7), DVE 53us (bn_stats/aggr, z+x2 PSUM evict-adds,
xT evict copies, 3/16 of the relus), Act 50us (relu 13/16, y2T evicts,
final out copies, sqrt), Pool/gpsimd 44us (both LN normalizes, x2+b2).
b2 and x2 are folded into the FFN2 PSUM accumulation via an identity-
weight matmul so the output eviction is a pure Act copy.

All big weights are pre-cast on the host and passed as fp8/bf16 DRAM
parameters (halves weight DMA traffic; no on-chip cast pass).
"""

import sys
from contextlib import ExitStack

for _p in ("/opt/trn_rl_repo", "/opt/pypackages"):
    if _p not in sys.path:
        sys.path.insert(0, _p)

import numpy as np
import ml_dtypes

import concourse.bass as bass
import concourse.mybir as mybir
import concourse.tile as tile
from concourse import bacc
from concourse.bass_utils import run_bass_kernel_spmd
from concourse.masks import make_identity

ALPHA, EPS = 0.9, 1e-5
S, B, D, FF = 2048, 16, 512, 2048
NCORES = 8
BL = B // NCORES            # batch items per core
T = 128                     # token tile
CB = 4                      # token tiles per chunk
NBLK = S // T               # 16
NCHUNK = NBLK // CB         # 4
NFT = FF // 128             # 16 f-tiles
KD = D // 128               # 4 d-tiles
NLAG = 1                    # decay lag blocks kept (lag>=2 < 2e-12 relative)
DVE_RELUS = (7, 15)          # these relu f-tiles run on DVE, rest on Act

F32 = mybir.dt.float32
F32R = mybir.dt.float32r
BF16 = mybir.dt.bfloat16
F8 = mybir.dt.float8e4
AF = mybir.ActivationFunctionType
ALU = mybir.AluOpType
DR = mybir.MatmulPerfMode.DoubleRow

NP_F8 = ml_dtypes.float8_e4m3
NP_BF16 = ml_dtypes.bfloat16


def _host_consts():
    """Decay-matrix derived constants, f64 -> f32 (mirrors reference)."""
    i = np.arange(S, dtype=np.float64)
    diff = i[:, None] - i[None, :]
    with np.errstate(under="ignore"):
        W = np.where(diff >= 0, ALPHA ** (diff + 1), 0.0)
        W = W + np.diag(1.0 - W.sum(axis=1))
        W = W.astype(np.float32)
        blocks = [
            np.ascontiguousarray(W[c * T : (c + 1) * T, c * T : (c + 1) * T].T)
            for c in range(NBLK)
        ]
        uniq, idx = [], []
        for blk in blocks:
            for j, u in enumerate(uniq):
                if np.array_equal(blk, u):
                    idx.append(j)
                    break
            else:
                idx.append(len(uniq))
                uniq.append(blk)
        wblkT = np.stack(uniq)  # [NU, T, T]
        lags = []
        for l in range(1, NLAG + 1):
            L = W[l * T : (l + 1) * T, 0:T]
            for i0 in range(l * T, S, T):
                assert np.array_equal(W[i0 : i0 + T, i0 - l * T : i0 - (l - 1) * T], L)
            lags.append(np.ascontiguousarray(L.T))
        wlagT = np.stack(lags)  # [NLAG, T, T]
    return wblkT.astype(np.float32), idx, wlagT.astype(np.float32)


_WBLKT, _BLKIDX, _WLAGT = _host_consts()
NU = _WBLKT.shape[0]

_NC_CACHE = {}


def build_nc():
    key = 0
    if key in _NC_CACHE:
        return _NC_CACHE[key]
    nc = bacc.Bacc()

    x_d = nc.declare_dram_parameter("x", [S, BL, D], F32, isOutput=False)
    wp_d = nc.declare_dram_parameter("wp", [D, D], BF16, isOutput=False)
    zb_d = nc.declare_dram_parameter("zb", [D], F32, isOutput=False)
    w1t_d = nc.declare_dram_parameter("w1t", [D, FF], F8, isOutput=False)
    hb_d = nc.declare_dram_parameter("hb", [FF], F32, isOutput=False)
    w2t_d = nc.declare_dram_parameter("w2t", [FF, D], F8, isOutput=False)
    b2_d = nc.declare_dram_parameter("b2", [D], F32, isOutput=False)
    wblk_d = nc.declare_dram_parameter("wblk", [NU, T, T], F32, isOutput=False)
    wlag_d = nc.declare_dram_parameter("wlag", [NLAG, T, T], F32, isOutput=False)
    out_d = nc.declare_dram_parameter("out", [S, BL, D], F32, isOutput=True)

    with tile.TileContext(nc) as tc, ExitStack() as ctx:
        pool = lambda name, bufs, **kw: ctx.enter_context(
            tc.tile_pool(name=name, bufs=bufs, **kw)
        )
        wgt = pool("wgt", 1)
        stage = pool("stage", 1)
        xin = pool("xin", 10)
        lnp = pool("ln", 5)
        yppp = pool("ypp", 4)
        xtp = pool("xt", 10)
        y2tp = pool("y2t", 3)
        zp = pool("z", 12)
        x2p = pool("x2", 10)
        hp = pool("h", 3)
        outp = pool("outp", 4)
        # single full-bank PSUM rotation; transposes write through bitcast
        # views so bf16/fp8 tiles share the same bank pool
        psmm = pool("psmm", 8, space="PSUM")

        # ---------------- one-time setup ----------------
        xpre = {}

        def preload_x(i, only=None):
            b, c = steps[i]
            tiles = xpre.get(i, [])
            for t in range(len(tiles), CB):
                if only is not None and t > only:
                    break
                s0 = (c * CB + t) * T
                xt = xin.tile([128, D], F32, tag="x")
                nc.sync.dma_start(xt[:], x_d.ap()[s0 : s0 + T, b, :])
                tiles.append(xt)
            xpre[i] = tiles

        # DMA order matters: the shared DMA device drains FIFO, so x(0) and
        # the weights the first iteration blocks on (zb, wp, wblk) go first.
        steps = [(b, c) for b in range(BL) for c in range(NCHUNK)]
        preload_x(0, only=1)  # first two x tiles, then weights step 0 blocks on
        # projection weight: bf16 straight from DRAM
        wp_r = wgt.tile([128, KD, D], BF16, tag="wp")
        nc.sync.dma_start(wp_r[:], wp_d.ap().rearrange("(kd p) e -> p kd e", p=128))
        zb_bc = wgt.tile([128, D], F32, tag="zb")
        nc.sync.dma_start(
            zb_bc[:], bass.AP(tensor=zb_d, offset=0, ap=[[0, 128], [1, D]])
        )
        preload_x(0)
        # mixing matrices: f32 DRAM -> resident f32r via casting DMA (SWDGE)
        wblk_r = wgt.tile([128, NU, T], F32R, tag="wblk")
        nc.gpsimd.dma_start(wblk_r[:], wblk_d.ap().rearrange("b j r -> j b r"))
        wlag_r = wgt.tile([128, NLAG, T], F32R, tag="wlag")
        nc.gpsimd.dma_start(wlag_r[:], wlag_d.ap().rearrange("b j r -> j b r"))
        preload_x(1)
        b2_bc = wgt.tile([128, D], F32, tag="b2")
        nc.sync.dma_start(
            b2_bc[:], bass.AP(tensor=b2_d, offset=0, ap=[[0, 128], [1, D]])
        )
        hb_sb = wgt.tile([128, NFT], F32, tag="hb")
        nc.sync.dma_start(
            hb_sb[:], bass.AP(tensor=hb_d, offset=0, ap=[[1, 128], [128, NFT]])
        )
        ident_f = stage.tile([128, 128], F32, tag="ident_f")
        make_identity(nc, ident_f[:])
        identB = wgt.tile([128, 128], BF16, tag="identB")
        nc.vector.tensor_copy(identB[:], ident_f[:])
        eps_t = wgt.tile([128, 1], F32, tag="eps")
        nc.vector.memset(eps_t[:], EPS)
        # tiny dummy activation: triggers the one-time activation-table load
        # while the pipeline is still waiting on the first x DMAs
        warm_t = wgt.tile([128, 1], F32, tag="warm")
        nc.scalar.activation(warm_t[:], eps_t[:], AF.Sqrt, bias=eps_t[:], scale=1.0)
        b2b = wgt.tile([128, D], BF16, tag="b2b")
        nc.vector.tensor_copy(b2b[:], b2_bc[:])

        # ---------------- helpers ----------------
        def ln_stats(xts, tag, per_tile=False):
            """4 tiles -> (mv4 [128,4,2], r4 [128,4]).

            batched (default): one sqrt+recip over all 4 tiles (fewer Act
            instrs); per_tile: sqrt/recip per tile so tile 0's normalize can
            start before tile 3's stats land (pipeline fill/drain).
            """
            mv4 = lnp.tile([128, CB, 2], F32, tag=f"mv4{tag}")
            r4 = lnp.tile([128, CB], F32, tag=f"r4{tag}")
            for t in range(CB):
                st = lnp.tile([128, 6], F32, tag=f"bnst{tag}")
                nc.vector.bn_stats(st[:], xts[t][:])
                nc.vector.bn_aggr(mv4[:, t, :], st[:])
                if per_tile:
                    nc.scalar.activation(
                        r4[:, t : t + 1], mv4[:, t, 1:2], AF.Sqrt,
                        bias=eps_t[:], scale=1.0,
                    )
                    nc.vector.reciprocal(r4[:, t : t + 1], r4[:, t : t + 1])
            if not per_tile:
                nc.scalar.activation(
                    r4[:], mv4[:, :, 1], AF.Sqrt, bias=eps_t[:], scale=1.0
                )
                nc.vector.reciprocal(r4[:], r4[:])
            return mv4, r4

        # ---------------- main pipeline ----------------
        zall = {b: [] for b in range(BL)}
        a_out, b_out = {}, {}

        def stage_ln1(i, per_tile=False, interleave_zev=False):
            """LN1 + transpose + projection, tile-interleaved on the PE."""
            b, c = steps[i]
            if i not in xpre:
                preload_x(i)
            xts = xpre.pop(i)
            if per_tile:
                # stats emitted inside the tile loop: tile t's full chain
                # completes while tile t+1's x DMA is still in flight
                mv4 = lnp.tile([128, CB, 2], F32, tag="mv4a")
                r4 = lnp.tile([128, CB], F32, tag="r4a")
            else:
                mv4, r4 = ln_stats(xts, "a")
            xT = []
            pzs = []

            def proj_tile(t):
                pz = psmm.tile([128, D], F32, tag="mm")
                for kd in range(KD):
                    nc.tensor.matmul(
                        pz[:],
                        xT[t][:, kd, :],
                        wp_r[:, kd, :],
                        start=(kd == 0),
                        stop=(kd == KD - 1),
                    )
                pzs.append(pz)
                if interleave_zev:
                    zt = zp.tile([128, D], F32R, tag="z")
                    nc.vector.tensor_add(zt[:], pz[:], zb_bc[:])
                    zall[b].append(zt)

            # one-tile lag: proj(t-1) is emitted after transpose(t), so the
            # PE runs proj(t-1) while tile t's evict copy (DVE) completes
            for t in range(CB):
                if per_tile:
                    st = lnp.tile([128, 6], F32, tag="bnsta")
                    nc.vector.bn_stats(st[:], xts[t][:])
                    nc.vector.bn_aggr(mv4[:, t, :], st[:])
                    nc.scalar.activation(
                        r4[:, t : t + 1], mv4[:, t, 1:2], AF.Sqrt,
                        bias=eps_t[:], scale=1.0,
                    )
                    nc.vector.reciprocal(r4[:, t : t + 1], r4[:, t : t + 1])
                ypp = yppp.tile([128, D], BF16, tag="ypp")
                nc.gpsimd.tensor_scalar(
                    out=ypp[:],
                    in0=xts[t][:],
                    scalar1=mv4[:, t, 0:1],
                    scalar2=r4[:, t : t + 1],
                    op0=ALU.subtract,
                    op1=ALU.mult,
                )
                ptb = psmm.tile([128, D], F32, tag="mm")
                pt = ptb[:].bitcast(BF16)  # [128, 1024] view, use cols 0..511
                for kd in range(KD):
                    nc.tensor.transpose(
                        pt[:, kd * 128 : (kd + 1) * 128],
                        ypp[:, kd * 128 : (kd + 1) * 128],
                        identB[:],
                    )
                xTt = xtp.tile([128, KD, 128], BF16, tag="xT")
                nc.vector.tensor_copy(
                    xTt[:], pt[:, 0:D].rearrange("p (a b) -> p a b", b=128)
                )
                xT.append(xTt)
                if t >= 1:
                    proj_tile(t - 1)
            proj_tile(CB - 1)
            a_out[i] = (xts, xT)
            b_out[i] = pzs

        def stage_zev(i):
            b, c = steps[i]
            for t in range(CB):
                zt = zp.tile([128, D], F32R, tag="z")
                nc.vector.tensor_add(zt[:], b_out[i][t][:], zb_bc[:])
                zall[b].append(zt)
            del b_out[i]

        stage_ln1(0, per_tile=True, interleave_zev=True)
        del b_out[0]

        # big fp8 weights: DMA'd in chunks interleaved with the early
        # pipeline so x loads aren't blocked behind the weight traffic.
        w1t_r = wgt.tile([128, KD, FF], F8, tag="w1t")
        w2t_r = wgt.tile([128, NFT, D], F8, tag="w2t")
        w1t_ap = w1t_d.ap().rearrange("(kd p) f -> p kd f", p=128)
        w2t_ap = w2t_d.ap().rearrange("(kf p) d -> p kf d", p=128)
        wload = [
            lambda kd2=kd2: nc.sync.dma_start(
                w1t_r[:, 2 * kd2 : 2 * kd2 + 2, :], w1t_ap[:, 2 * kd2 : 2 * kd2 + 2, :]
            )
            for kd2 in range(KD // 2)
        ] + [
            lambda f8=f8: nc.sync.dma_start(
                w2t_r[:, 8 * f8 : 8 * f8 + 8, :], w2t_ap[:, 8 * f8 : 8 * f8 + 8, :]
            )
            for f8 in range(2)
        ]
        wload.reverse()  # pop() from the front
        wload.pop()()

        def ffn1_part(y2T, ht, fts, dve_relus=DVE_RELUS):
            """fp8 DoubleRow FFN1 + relu (split Act/DVE) for given f-tiles."""
            for ft in fts:
                ph = psmm.tile([128, CB * 128], F32, tag="mm")
                for j in range(KD // 2):
                    nc.tensor.matmul(
                        ph[:],
                        w1t_r[:, 2 * j : 2 * j + 2, ft * 128 : (ft + 1) * 128],
                        y2T[:, 2 * j : 2 * j + 2, :],
                        start=(j == 0),
                        stop=(j == KD // 2 - 1),
                        perf_mode=DR,
                    )
                if ft in dve_relus:
                    nc.vector.tensor_scalar(
                        out=ht[:, ft, :],
                        in0=ph[:],
                        scalar1=hb_sb[:, ft : ft + 1],
                        scalar2=0.0,
                        op0=ALU.add,
                        op1=ALU.max,
                    )
                else:
                    nc.scalar.activation(
                        ht[:, ft, :],
                        ph[:],
                        AF.Relu,
                        bias=hb_sb[:, ft : ft + 1],
                        scale=1.0,
                    )

        def ffn2_part(ht, x2pbs, b, c, ts):
            """fp8 DoubleRow FFN2 + (x2+b2) via identity matmul -> out DMA."""
            for t in ts:
                s0 = (c * CB + t) * T
                po = psmm.tile([128, D], F32, tag="mm")
                for j in range(NFT // 2):
                    nc.tensor.matmul(
                        po[:],
                        ht[:, 2 * j : 2 * j + 2, t * 128 : (t + 1) * 128],
                        w2t_r[:, 2 * j : 2 * j + 2, :],
                        start=(j == 0),
                        stop=False,
                        perf_mode=DR,
                    )
                nc.tensor.matmul(
                    po[:], identB[:], x2pbs[t][:], start=False, stop=True
                )
                ot = outp.tile([128, D], F32, tag="o")
                nc.scalar.activation(ot[:], po[:], AF.Copy)
                nc.sync.dma_start(out_d.ap()[s0 : s0 + T, b, :], ot[:])

        # Software pipeline, one-step-deep FFN deferral: iteration i runs
        # step i's mixing/LN2 and step i+1's LN1/projection, with step i-1's
        # FFN matmuls emitted where the PE would otherwise stall on the LN
        # stat chains (DVE/Act/Pool latency).
        ffn_prev = None
        for i, (b, c) in enumerate(steps):
            xts, _ = a_out.pop(i)
            # --- mixing (banded decay matmul) ---
            pms = []
            for t in range(CB):
                blk = c * CB + t
                nmix = 1 + min(blk, NLAG)
                pm = psmm.tile([128, D], F32, tag="mm")
                nc.tensor.matmul(
                    pm[:],
                    wblk_r[:, _BLKIDX[blk], :],
                    zall[b][blk][:],
                    start=True,
                    stop=(nmix == 1),
                )
                for l in range(1, nmix):
                    nc.tensor.matmul(
                        pm[:],
                        wlag_r[:, l - 1, :],
                        zall[b][blk - l][:],
                        start=False,
                        stop=(l == nmix - 1),
                    )
                pms.append(pm)
            if i + 1 < len(steps):
                preload_x(i + 1)
            # --- x2 = x + attn (DVE, bf16) with tile-interleaved LN2 stats
            # (per-tile sqrt/recip so norm2(t0) starts ~4us earlier than a
            # batched chain would allow) ---
            x2ts = []
            mv4b = lnp.tile([128, CB, 2], F32, tag="mv4b")
            r4b = lnp.tile([128, CB], F32, tag="r4b")
            for t in range(CB):
                x2t = x2p.tile([128, D], BF16, tag="x2")
                nc.vector.tensor_add(x2t[:], pms[t][:], xts[t][:])
                x2ts.append(x2t)
                st = lnp.tile([128, 6], F32, tag="bnstb")
                nc.vector.bn_stats(st[:], x2t[:])
                nc.vector.bn_aggr(mv4b[:, t, :], st[:])
                nc.scalar.activation(
                    r4b[:, t : t + 1], mv4b[:, t, 1:2], AF.Sqrt,
                    bias=eps_t[:], scale=1.0,
                )
                nc.vector.reciprocal(r4b[:, t : t + 1], r4b[:, t : t + 1])
            # --- LN2 per-tile emitter (norm2 Pool, T2 PE, evict Act/DVE) ---
            y2T = y2tp.tile([128, KD, CB * 128], F8, tag="y2T")
            y2pps = {}

            def ln2_norm(t):
                # bf16 through the transpose (the BIR verifier rejects fp8
                # transpose outputs with unit stride); fp8 cast happens at
                # the eviction copy
                y2pp = yppp.tile([128, D], BF16, tag="y2pp")
                nc.gpsimd.tensor_scalar(
                    out=y2pp[:],
                    in0=x2ts[t][:],
                    scalar1=mv4b[:, t, 0:1],
                    scalar2=r4b[:, t : t + 1],
                    op0=ALU.subtract,
                    op1=ALU.mult,
                )
                y2pps[t] = y2pp

            def ln2_tile(t):
                if t not in y2pps:
                    ln2_norm(t)
                y2pp = y2pps.pop(t)
                pt2b = psmm.tile([128, D], F32, tag="mm")
                pt2 = pt2b[:].bitcast(BF16)  # [128, 1024] view, use cols 0..511
                for kd in range(KD):
                    nc.tensor.transpose(
                        pt2[:, kd * 128 : (kd + 1) * 128],
                        y2pp[:, kd * 128 : (kd + 1) * 128],
                        identB[:],
                    )
                if i + 1 == len(steps):
                    # last iteration: Act is clogged with this iteration's
                    # relus/outs -- evict on (idle) DVE so the epilogue FFN
                    # isn't stalled behind them
                    nc.vector.tensor_copy(
                        y2T[:, :, t * 128 : (t + 1) * 128],
                        pt2[:, 0:D].rearrange("p (a b) -> p a b", b=128),
                    )
                else:
                    nc.scalar.activation(
                        y2T[:, :, t * 128 : (t + 1) * 128],
                        pt2[:, 0:D].rearrange("p (a b) -> p a b", b=128),
                        AF.Copy,
                    )

            # --- previous step's FFN: chunky ready-to-run PE work covering
            # this step's LN2 chain and next step's LN1 chain ---
            if ffn_prev is not None:
                ht_prev = hp.tile([128, NFT, CB * 128], F8, tag="h")
                ffn1_part(ffn_prev[0], ht_prev, range(NFT))
                if i + 1 == len(steps):
                    # last iteration: run LN2 (whose evicts gate the epilogue
                    # FFN) before ffn2 so the DVE evicts overlap it
                    for t in range(CB):
                        ln2_tile(t)
                    ffn2_part(ht_prev, ffn_prev[1], ffn_prev[2], ffn_prev[3],
                              range(CB))
                else:
                    ffn2_part(ht_prev, ffn_prev[1], ffn_prev[2], ffn_prev[3],
                              range(CB))
                    for t in range(CB):
                        ln2_tile(t)
            else:
                # iteration 0 has no deferred FFN to cover the LN2 chain;
                # run the next step's LN1/projection there instead. The LN2
                # normalizes go first on the Pool queue (they gate T2), while
                # the PE queue gets T1/proj before T2.
                for t in range(CB):
                    ln2_norm(t)
                if i + 1 < len(steps):
                    stage_ln1(i + 1, per_tile=True)
                for t in range(CB):
                    ln2_tile(t)
            if wload:
                wload.pop()()
            # --- next step's LN1 ---
            if ffn_prev is not None and i + 1 < len(steps):
                stage_ln1(i + 1)
            # x2+b2 (Pool) late: only needed by next iteration's ffn2
            x2pbs = []
            for t in range(CB):
                x2pb = x2p.tile([128, D], BF16, tag="x2pb")
                nc.gpsimd.tensor_add(x2pb[:], x2ts[t][:], b2b[:])
                x2pbs.append(x2pb)
            if wload:
                wload.pop()()
            if i + 1 < len(steps):
                stage_zev(i + 1)
            if wload:
                wload.pop()()
            ffn_prev = (y2T, x2pbs, b, c)
        # epilogue: last step's FFN; relu split 50/50 so neither engine's
        # queue becomes the drain tail
        ht_prev = hp.tile([128, NFT, CB * 128], F8, tag="h")
        ffn1_part(ffn_prev[0], ht_prev, range(NFT),
                  dve_relus=(1, 3, 5, 7, 9, 11, 13, 15))
        ffn2_part(ht_prev, ffn_prev[1], ffn_prev[2], ffn_prev[3], range(CB))

    nc.compile()
    _NC_CACHE[key] = nc
    return nc


def _prep_inputs(x, w_lin, b_lin, w1, b1, w2, b2, g1, beta1, g2, beta2):
    f32 = np.float32
    wp = np.ascontiguousarray(w_lin.T * g1[:, None]).astype(NP_BF16)
    zb = (w_lin.astype(np.float64) @ beta1.astype(np.float64) + b_lin).astype(f32)
    w1t = np.ascontiguousarray(w1.T * g2[:, None]).astype(NP_F8)
    hb = (w1.astype(np.float64) @ beta2.astype(np.float64) + b1).astype(f32)
    w2t = np.ascontiguousarray(w2.T).astype(NP_F8)
    shared = {
        "wp": wp,
        "zb": zb,
        "w1t": w1t,
        "hb": hb,
        "w2t": w2t,
        "b2": b2.astype(f32),
        "wblk": _WBLKT,
        "wlag": _WLAGT,
    }
    in_maps = []
    for cc in range(NCORES):
        m = dict(shared)
        m["x"] = np.ascontiguousarray(x[:, cc * BL : (cc + 1) * BL, :]).astype(f32)
        in_maps.append(m)
    return in_maps


def kernel(**inputs):
    nc = build_nc()
    in_maps = _prep_inputs(**inputs)
    res = run_bass_kernel_spmd(nc, in_maps, list(range(NCORES)))
    out = np.concatenate([r["out"] for r in res.results], axis=1)
    return out.astype(np.float32)


if __name__ == "__main__":
    rng = np.random.default_rng(0)
    demo = {
        "x": rng.standard_normal((S, B, D)).astype(np.float32),
        "w_lin": rng.standard_normal((D, D)).astype(np.float32) * D**-0.5,
        "b_lin": rng.standard_normal((D,)).astype(np.float32) * 0.01,
        "w1": rng.standard_normal((FF, D)).astype(np.float32) * D**-0.5,
        "b1": rng.standard_normal((FF,)).astype(np.float32) * 0.01,
        "w2": rng.standard_normal((D, FF)).astype(np.float32) * FF**-0.5,
        "b2": rng.standard_normal((D,)).astype(np.float32) * 0.01,
        "g1": np.ones(D, np.float32),
        "beta1": np.zeros(D, np.float32),
        "g2": np.ones(D, np.float32),
        "beta2": np.zeros(D, np.float32),
    }
    out = kernel(**demo)
    print("ok", out.shape, out.dtype)
